# revision 1
# baseline (speedup 1.0000x reference)
"""Trainium2 Bass kernel for the digit-conv model.

Math: y = relu(relu(conv3x3(x) @ W1 + b1) @ W2 + b2) @ W3 + b3.
The valid 3x3 conv is linear, so it folds into W1 on device:
feat = x @ A with A[u, q] sparse from conv_w, hence
W1eff = A @ W1 and y = mlp(x @ W1eff ...). The kernel computes
W1eff = A^T.T @ W1 on the tensor engine once (A^T is banded, so
all-zero blocks are statically skipped), then streams the batch
through the 3-layer MLP entirely as lhsT.T @ rhs matmuls with channels
on partitions and batch on the free dimension (no transposes needed:
x is supplied pre-transposed per shard, and every weight is already in
[K, M] layout).

Sharding: pure data parallel — batch split across 8 cores, weights
replicated. Host-side work is limited to layout (x transpose + shard +
bf16 cast, zero-padding, band extraction) and scattering the 9 conv
weights into the A^T matrix (no arithmetic).

All matmul operands are bf16: fp32 x (25.7 MB/core) would be DMA-bound
(~250 GB/s/core effective -> ~103 us, above the ~97 us tensor-engine
floor). PSUM accumulation is fp32; biases are applied in fp32 from
PSUM. (f32r was measured more accurate but cannot be mixed with bf16:
walrus inserts round-to-fp32r passes over DMA-loaded f32r tiles that
corrupt neighboring tiles in a mixed-dtype program.)

DMA instruction count is minimized (one multi-tile DMA per logical
group via 3D access patterns): each dma_start costs ~650 ns of serial
issue on its queue engine, which dominated startup in earlier
revisions. The packed fold tensor and x supers go on the Sync (HWDGE)
path; small constants (w2/w3/bias) go through GpSimd (SWDGE) so they
don't serialize with them. Full-width (N=512) dummy matmuls on a
memset tile warm the PE clock-gate before real work arrives — HAM
watches array-busy duty cycle, so narrow warmup matmuls do NOT trip
it. The batch super-block widths ramp 256 -> 1536 so the main loop
starts right behind the fold DMA, and the final 256-wide super halves
the exposed end-of-stream dependency chain. A small post-fold filler
block of dummy matmuls bridges the fold->x_0 DMA wait so an unlucky
per-core HAM window phase cannot re-throttle the clock (this cut the
worst-core exec from ~119 to ~116 us; per-core spread ~113.5-116.3).

Measured on the 8 axon TRN2 cores: ~113-118 us HW exec at 2.4 GHz
(the chip drifts to a 2.0 GHz power state at times: +20%), ~5.2e-3
scale-relative error vs the fp32 reference. Breakdown: ~7 us fixed
NEFF preamble, ~5 us DMA-bound ramp-in (fold + first x super),
~93 us matmul stream with <1.5 us of gaps (400 main matmuls x 216 ns
is the hard floor: ceil(784/128)*ceil(300/128)+ceil(300/128)+1 = 25
bank passes per 512 batch columns), ~11 us fixed walrus
semaphore-teardown and drain barrier.
"""

import ml_dtypes
import numpy as np

import concourse.tile as tile
from concourse import bacc, mybir
from concourse import bass_utils

N_CORES = 8
B = 65536
BC = B // N_CORES  # 8192 rows per core
U = 784            # input features (28*28)
Q = 676            # conv outputs (26*26)
QP = 768           # q padded to 6 full tiles of 128
H1, H2, H3 = 300, 100, 10
NB = 512           # batch columns per PSUM block (one bank of fp32)
SUP = 1536         # max batch columns per DMA super-block
SUP_WIDTHS = [256, 512, 1024, 1536, 1536, 1536, 1536, 256]
assert sum(SUP_WIDTHS) == BC
KT = 112           # u-dim k-tile (784 = 7*112)
NKT = 7
MC = 100           # layer-1 output chunk (300 = 3*100)
NMC = 3
ABW = 336          # amat band width (3 u-chunks), fixed for all q-tiles

_prog_cache = {}


def _fold_bands():
    """Static block-sparsity of A^T [Q, U]: per 128-row q-tile, the nonzero
    columns lie in a band; returns per-tile (q0, p_real, c_lo, c_hi) with the
    band given in whole 112-wide u-chunks (at most 3 chunks wide)."""
    bands = []
    for qt in range(QP // 128):
        q0 = qt * 128
        p_real = min(128, Q - q0)
        i_lo = q0 // 26
        i_hi = (q0 + p_real - 1) // 26
        u_lo = 28 * i_lo
        u_hi = min(U, 28 * (i_hi + 2) + 28)   # exclusive upper bound
        c_lo = u_lo // KT
        c_hi = (u_hi + KT - 1) // KT          # exclusive chunk bound
        assert c_hi - c_lo <= ABW // KT
        bands.append((q0, p_real, c_lo, c_hi))
    return bands


def _build_program():
    f32 = mybir.dt.float32
    bf16 = mybir.dt.bfloat16
    relu = mybir.ActivationFunctionType.Relu
    alu_add = mybir.AluOpType.add
    alu_max = mybir.AluOpType.max

    nc = bacc.Bacc(
        "TRN2", target_bir_lowering=False, debug=False, num_devices=N_CORES
    )

    nqt = QP // 128
    xT_d = nc.dram_tensor("xT", [U, BC], bf16, kind="ExternalInput").ap()
    FW = ABW + H1  # 636: packed [amat band | w1] row width
    fold_d = nc.dram_tensor("fold", [QP, FW], bf16, kind="ExternalInput").ap()
    w2_d = nc.dram_tensor("w2", [H1, H2], bf16, kind="ExternalInput").ap()
    w3_d = nc.dram_tensor("w3", [H2, H3], bf16, kind="ExternalInput").ap()
    bias_d = nc.dram_tensor("bias", [MC, 5], f32, kind="ExternalInput").ap()
    yT_d = nc.dram_tensor("yT", [H3, BC], f32, kind="ExternalOutput").ap()

    bands = _fold_bands()

    with tile.TileContext(nc) as tc:
        with tc.tile_pool(name="const", bufs=1) as cpool, \
             tc.tile_pool(name="xp", bufs=5) as xpool, \
             tc.tile_pool(name="hp", bufs=2) as hpool, \
             tc.tile_pool(name="yp", bufs=2) as ypool, \
             tc.tile_pool(name="ps1", bufs=6, space="PSUM") as ps1p, \
             tc.tile_pool(name="ps2", bufs=2, space="PSUM") as ps2p:

            # ---- HAM warmup: dummy matmuls on a memset tile so the PE
            # clock-gate releases before the real work arrives ----
            warm_sb = cpool.tile([128, 512], bf16)
            nc.vector.memset(warm_sb[:], 0.0)
            for wi in range(15):
                pw = ps2p.tile([128, NB], f32, tag="l2", name=f"pwarm_{wi}")
                nc.tensor.matmul(pw[:], warm_sb[:, :128], warm_sb[:],
                                 start=True, stop=True)

            # ---- constants into SBUF (one merged DMA per group, on the
            # SWDGE path so they don't block x-load issue on HWDGE) ----
            fold_sb = cpool.tile([128, nqt * FW], bf16)
            nc.sync.dma_start(
                fold_sb[:].rearrange("p (q c) -> p q c", c=FW),
                fold_d.rearrange("(q p) c -> p q c", p=128),
            )
            w2_sb = cpool.tile([MC, NMC * H2], bf16)
            nc.gpsimd.dma_start(
                w2_sb[:].rearrange("p (k c) -> p k c", c=H2),
                w2_d.rearrange("(k p) c -> p k c", p=MC),
            )
            w3_sb = cpool.tile([H2, H3], bf16)
            nc.gpsimd.dma_start(w3_sb[:], w3_d)
            bias_sb = cpool.tile([MC, 5], f32)
            nc.gpsimd.dma_start(bias_sb[:], bias_d)

            # ---- fold the conv into W1: W1eff[u, c] = (A^T).T @ W1 ----
            # Only q-tiles whose band covers the u-chunk contribute; the
            # rest are all-zero blocks of the banded A^T and are skipped.
            # (fold PSUM shares the l1 slot group: same tag, bank-sized)
            w1eff_sb = cpool.tile([KT, NKT * H1], bf16)
            for ut in range(NKT):
                parts = [qt for qt, (_, _, c_lo, c_hi) in enumerate(bands)
                         if c_lo <= ut < c_hi]
                assert parts
                pf = ps1p.tile([KT, NB], f32, tag="l1", name=f"pfold_{ut}")
                for idx, qt in enumerate(parts):
                    _, _, c_lo, _ = bands[qt]
                    off = qt * FW + (ut - c_lo) * KT
                    nc.tensor.matmul(
                        pf[:, :H1],
                        fold_sb[:, off:off + KT],
                        fold_sb[:, qt * FW + ABW:(qt + 1) * FW],
                        start=(idx == 0),
                        stop=(idx == len(parts) - 1),
                    )
                nc.vector.tensor_copy(
                    w1eff_sb[:, ut * H1:(ut + 1) * H1], pf[:, :H1])

            # ---- post-fold filler: bridge the fold->x_0 DMA wait so an
            # unlucky HAM MID-window phase can't re-throttle the PE on
            # cores whose x ramp lands late ----
            for wi in range(4):
                pw = ps2p.tile([128, NB], f32, tag="l2", name=f"pfill_{wi}")
                nc.tensor.matmul(pw[:], warm_sb[:, :128], warm_sb[:],
                                 start=True, stop=True)

            # ---- main pipeline over batch super-blocks ----
            # L2/L3 of each block are emitted AFTER the next block's
            # L1+relu, so the PE reaches them with their ACT dependencies
            # long satisfied (removes ~80ns stalls at the chunk->L2 edge).
            def emit_l2l3(h1_sb, nb, y_sb, pb, y_start, y_sw, last_of_sup):
                p2 = ps2p.tile([H2, nb], f32, tag="l2",
                               name=f"p2_{h1_sb.tensor.name}",
                               padded_shape=[H2, NB])
                for k2 in range(3):
                    nc.tensor.matmul(
                        p2[:], w2_sb[:, k2 * H2:(k2 + 1) * H2],
                        h1_sb[:, k2 * nb:(k2 + 1) * nb],
                        start=(k2 == 0), stop=(k2 == 2),
                    )
                h2 = hpool.tile([H2, nb], bf16, tag="h2",
                                name=f"h2_{h1_sb.tensor.name}",
                                padded_shape=[H2, NB])
                nc.vector.tensor_scalar(
                    h2[:], p2[:], bias_sb[:, 3:4], 0.0, alu_add, alu_max
                )
                p3 = ps2p.tile([H3, nb], f32, tag="l2",
                               name=f"p3_{h1_sb.tensor.name}",
                               padded_shape=[H3, NB])
                nc.tensor.matmul(p3[:], w3_sb[:], h2[:],
                                 start=True, stop=True)
                nc.vector.tensor_scalar_add(
                    y_sb[:, pb * NB:pb * NB + nb], p3[:],
                    bias_sb[:H3, 4:5])
                if last_of_sup:
                    nc.sync.dma_start(
                        yT_d[:, y_start:y_start + y_sw], y_sb[:])

            pending = None
            sup_start = 0
            for sup, sw in enumerate(SUP_WIDTHS):
                xtile = xpool.tile([KT, NKT * sw], bf16, tag="x",
                                   name=f"xt_{sup}",
                                   padded_shape=[KT, NKT * SUP])
                nc.sync.dma_start(
                    xtile[:].rearrange("p (k c) -> p k c", c=sw),
                    xT_d[:, sup_start:sup_start + sw]
                    .rearrange("(k p) c -> p k c", p=KT),
                )

                y_sb = ypool.tile([H3, sw], f32, tag="y", name=f"y_{sup}",
                                  padded_shape=[H3, SUP])
                for pb in range((sw + NB - 1) // NB):
                    nb = min(NB, sw - pb * NB)
                    h1_sb = hpool.tile([MC, NMC * nb], bf16, tag="h1",
                                       name=f"h1_{sup}_{pb}",
                                       padded_shape=[MC, NMC * NB])
                    for mc in range(NMC):
                        p1 = ps1p.tile([MC, nb], f32, tag="l1",
                                       name=f"p1_{sup}_{pb}_{mc}",
                                       padded_shape=[MC, NB])
                        for kt in range(NKT):
                            nc.tensor.matmul(
                                p1[:],
                                w1eff_sb[:, kt * H1 + mc * MC:
                                         kt * H1 + (mc + 1) * MC],
                                xtile[:, kt * sw + pb * NB:
                                      kt * sw + pb * NB + nb],
                                start=(kt == 0),
                                stop=(kt == NKT - 1),
                            )
                        nc.scalar.activation(
                            h1_sb[:, mc * nb:(mc + 1) * nb], p1[:], relu,
                            bias=bias_sb[:, mc:mc + 1], scale=1.0,
                        )

                    if pending is not None:
                        emit_l2l3(*pending)
                    nblocks = (sw + NB - 1) // NB
                    pending = (h1_sb, nb, y_sb, pb, sup_start, sw,
                               pb == nblocks - 1)

                sup_start += sw

            emit_l2l3(*pending)

    nc.compile()
    return nc


def _build_amat_banded(conv_w: np.ndarray) -> np.ndarray:
    """Scatter the 9 conv weights into the banded A^T [QP, ABW]:
    A^T[q, u] = conv_w[ki, kj] for q = 26*i + j, u = 28*(i+ki) + (j+kj),
    stored per 128-row q-tile with columns [c_lo*KT, c_hi*KT) of the band."""
    amat = np.zeros((Q, U), np.float32)
    i = np.arange(26)
    j = np.arange(26)
    q = (26 * i[:, None] + j[None, :]).ravel()
    for ki in range(3):
        for kj in range(3):
            u = (28 * (i[:, None] + ki) + j[None, :] + kj).ravel()
            amat[q, u] = conv_w[ki, kj]
    banded = np.zeros((QP, ABW), np.float32)
    for (q0, p_real, c_lo, c_hi) in _fold_bands():
        w = (c_hi - c_lo) * KT
        banded[q0:q0 + p_real, :w] = amat[q0:q0 + p_real, c_lo * KT:c_hi * KT]
    return banded


def _make_in_maps(x, conv_w, W1, b1, W2, b2, W3, b3):
    bf = ml_dtypes.bfloat16
    xT = np.ascontiguousarray(x.T.astype(bf))  # [U, B] bf16
    foldpk = np.zeros((QP, ABW + H1), np.float32)
    foldpk[:, :ABW] = _build_amat_banded(conv_w)
    foldpk[:Q, ABW:] = np.asarray(W1, np.float32)
    foldpk = np.ascontiguousarray(foldpk.astype(bf))
    w2 = np.ascontiguousarray(np.asarray(W2, np.float32).astype(bf))
    w3 = np.ascontiguousarray(np.asarray(W3, np.float32).astype(bf))
    bias = np.zeros((MC, 5), np.float32)
    bias[:, :NMC] = np.asarray(b1, np.float32).reshape(NMC, MC).T
    bias[:, 3] = np.asarray(b2, np.float32)
    bias[:H3, 4] = np.asarray(b3, np.float32)
    in_maps = []
    for c in range(N_CORES):
        in_maps.append({
            "xT": np.ascontiguousarray(xT[:, c * BC:(c + 1) * BC]),
            "fold": foldpk,
            "w2": w2, "w3": w3,
            "bias": bias,
        })
    return in_maps


def kernel(x, conv_w, W1, b1, W2, b2, W3, b3):
    x = np.asarray(x, dtype=np.float32)
    conv_w = np.asarray(conv_w, dtype=np.float32)

    if "nc" not in _prog_cache:
        _prog_cache["nc"] = _build_program()
    nc = _prog_cache["nc"]

    in_maps = _make_in_maps(x, conv_w, W1, b1, W2, b2, W3, b3)
    res = bass_utils.run_bass_kernel_spmd(
        nc, in_maps, core_ids=list(range(N_CORES))
    )

    out = np.empty((B, H3), np.float32)
    for c in range(N_CORES):
        out[c * BC:(c + 1) * BC, :] = res.results[c]["yT"].T
    return out



# revision 3
# speedup vs baseline: 1.0312x; 1.0312x over previous
"""Trainium2 Bass kernel for the digit-conv model, v2 (tile-packed).

Math: y = relu(relu(conv3x3(x) @ W1 + b1) @ W2 + b2) @ W3 + b3.
The valid 3x3 conv folds into W1 on device (W1eff = A @ W1 with banded
A^T, computed once on the tensor engine), so the stream is a 3-layer
MLP with channels on partitions and batch on the free dimension.

v2 recovers the ragged-tile waste of v1 (which ran 25 N=512 passes per
512-batch block) with PE array tiling:
  - L1 k-tiles of 128 (6 full + K=16 tail), m-chunks {128, 128, 44}.
    The 12 (m0,m1)x(6 k) passes use the full 128x128 array.
  - The K=16 tail for m0+m1 runs as one 32x128-mode row-tiled span
    (2 concurrent MMs on row strips; even blocks use strips 0,1, odd
    blocks strips 2,3 so a block pair's 4 tail MMs form one span).
    The tail x rows are DMA-replicated to partition groups 0/32/64/96.
  - The M=44 chunk (mt) runs col-paired in 128x64 mode: k-tiles
    {0,2,4,6} accumulate at PSUM parts 0:44 (T0) while {1,3,5} run
    concurrently at parts 64:108 (T1); a cross-partition DVE add
    combines the halves before the relu. The 7th k-tile rides in pa's
    chain as a zero-padded K=128 MM (w1eff6 rows 16:128 are zero).
  - L3 (K=100, M=10) col-packs 4 blocks into one 128x32-mode span
    (tile_position (0,32j), PSUM partition slices 32j:32j+10).
Per 512-block: 17 L1 spans + 3 L2 passes + 0.25 L3 spans vs 25.
Same-mode work is grouped per block pair (S1 tail span, S2 full
passes + L2, S3 mt spans, L3 every other pair) to amortize tiling
mode-switch drains.

All matmul operands bf16 (fp8 DoubleRow fails the 2e-2 gate: measured
4.5e-2), PSUM fp32, biases applied from PSUM in fp32. Fold DMA goes on
the scalar HWDGE queue, x supers + tail replicas on sync (tails first
so the S1 spans don't wait on the big x transfer), small consts and y
outputs on gpsimd SWDGE. HAM warmup + post-fold filler as in v1.
"""

import ml_dtypes
import numpy as np

import concourse.tile as tile
from concourse import bacc, mybir
from concourse import bass_utils

N_CORES = 8
B = 65536
BC = B // N_CORES  # 8192 rows per core
U = 784            # input features (28*28)
Q = 676            # conv outputs (26*26)
QP = 768           # q padded to 6 tiles of 128
H1, H2, H3 = 300, 100, 10
NB = 512           # batch columns per block (one PSUM bank of fp32)
KT = 128           # u-dim k-tile
NKT = 6            # full k-tiles
KTAIL = U - NKT * KT   # 16
MT = 44            # ragged m-chunk width (300 - 2*128)
MTH = 22           # mt half-chunk (one 128x32 col tile per half)
ABW = 384          # banded A^T width: 3 chunks of 128
FW = ABW + H1      # 684 packed fold row width
SUP = 1536         # max batch columns per DMA super-block
SUP_WIDTHS = [512, 1024, 1536, 1536, 1536, 1536, 512]
NBLK = 16
assert sum(SUP_WIDTHS) == BC and NBLK * NB == BC

_SUP_START = [0]
for _w in SUP_WIDTHS:
    _SUP_START.append(_SUP_START[-1] + _w)
_BLK2SUP = []  # block -> (super, col offset within super)
for _s, _w in enumerate(SUP_WIDTHS):
    for _off in range(0, _w, NB):
        _BLK2SUP.append((_s, _off))
assert len(_BLK2SUP) == NBLK

_prog_cache = {}


def _fold_bands():
    """Static block-sparsity of A^T [Q, U]: per 128-row q-tile the nonzero
    columns lie in a band; band given in whole 128-wide u-chunks."""
    bands = []
    for t in range(6):
        q0 = t * 128
        p_real = min(128, Q - q0)
        i_lo = q0 // 26
        i_hi = (q0 + p_real - 1) // 26
        u_lo = 28 * i_lo
        u_hi = min(U, 28 * (i_hi + 3))
        c_lo = u_lo // 128
        c_hi = -(-u_hi // 128)
        assert c_hi - c_lo <= 3
        bands.append((q0, p_real, c_lo, c_hi))
    return bands


def _build_program():
    f32 = mybir.dt.float32
    bf16 = mybir.dt.bfloat16
    relu = mybir.ActivationFunctionType.Relu
    alu_add = mybir.AluOpType.add
    alu_max = mybir.AluOpType.max

    nc = bacc.Bacc(
        "TRN2", target_bir_lowering=False, debug=False, num_devices=N_CORES
    )

    xT_d = nc.dram_tensor("xT", [U, BC], bf16, kind="ExternalInput").ap()
    fold_d = nc.dram_tensor("fold", [QP, FW], bf16, kind="ExternalInput").ap()
    w2_d = nc.dram_tensor("w2", [4 * KT, H2], bf16, kind="ExternalInput").ap()
    w3_d = nc.dram_tensor("w3", [H2, H3], bf16, kind="ExternalInput").ap()
    bias_d = nc.dram_tensor("bias", [128, 5], f32, kind="ExternalInput").ap()
    yT_d = nc.dram_tensor("yT", [H3, BC], f32, kind="ExternalOutput").ap()

    bands = _fold_bands()
    cover = [[t for t, (_, _, cl, ch) in enumerate(bands) if cl <= ut < ch]
             for ut in range(7)]

    with tile.TileContext(nc) as tc:
        with tc.tile_pool(name="const", bufs=1) as cpool, \
             tc.tile_pool(name="xp", bufs=5) as xpool, \
             tc.tile_pool(name="hp", bufs=3) as hpool, \
             tc.tile_pool(name="h2p", bufs=5) as h2pool, \
             tc.tile_pool(name="yp", bufs=2) as ypool, \
             tc.tile_pool(name="ps1", bufs=4, space="PSUM") as ps1p, \
             tc.tile_pool(name="psmt", bufs=2, space="PSUM") as psmtp, \
             tc.tile_pool(name="ps2", bufs=2, space="PSUM") as ps2p:

            # ---- HAM warmup ----
            warm_sb = cpool.tile([128, 512], bf16)
            nc.vector.memset(warm_sb[:], 0.0)
            for wi in range(15):
                pw = psmtp.tile([128, NB], f32, tag="mt", name=f"pwarm_{wi}")
                nc.tensor.matmul(pw[:], warm_sb[:, :128], warm_sb[:],
                                 start=True, stop=True)

            # ---- constants ----
            fold_sb = cpool.tile([128, 6 * FW], bf16)
            nc.scalar.dma_start(
                fold_sb[:].rearrange("p (q c) -> p q c", c=FW),
                fold_d.rearrange("(q p) c -> p q c", p=128),
            )
            w2_sb = cpool.tile([128, 4 * H2], bf16)
            nc.gpsimd.dma_start(
                w2_sb[:].rearrange("p (k c) -> p k c", c=H2),
                w2_d.rearrange("(k p) c -> p k c", p=128),
            )
            w3_sb = cpool.tile([H2, H3], bf16)
            nc.gpsimd.dma_start(w3_sb[:], w3_d)
            bias_sb = cpool.tile([128, 5], f32)
            nc.gpsimd.dma_start(bias_sb[:], bias_d)

            # ---- fold the conv into W1: W1eff[u, c] = (A^T).T @ W1 ----
            w1eff_sb = cpool.tile([128, NKT * H1], bf16)
            w1eff6_sb = cpool.tile([128, H1], bf16)
            nc.vector.memset(w1eff6_sb[:], 0.0)
            fv = fold_sb[:].rearrange("p (q c) -> p q c", c=FW)
            for ut in range(NKT):
                pf = ps1p.tile([128, H1], f32, tag="l1", name=f"pfold_{ut}",
                               padded_shape=[128, NB])
                parts = cover[ut]
                for idx, t in enumerate(parts):
                    c_lo = bands[t][2]
                    off = (ut - c_lo) * 128
                    nc.tensor.matmul(
                        pf[:], fv[:, t, off:off + 128], fv[:, t, ABW:FW],
                        start=(idx == 0), stop=(idx == len(parts) - 1))
                nc.vector.tensor_copy(
                    w1eff_sb[:, ut * H1:(ut + 1) * H1], pf[:])
            # tail chunk (K=16): produce the tail weights at partition
            # groups 0/32/64/96 via col-tiled fold MMs (no cross-partition
            # copies needed): parts 0:16 all 300 cols (m0 even + mt),
            # 32:48 m1-even cols, 64:80 m0-odd cols, 96:112 m1-odd cols.
            t5 = cover[6][0]
            c_lo5 = bands[t5][2]
            lhs6 = fv[:, t5, (6 - c_lo5) * 128:(6 - c_lo5) * 128 + KTAIL]
            pf6 = ps1p.tile([128, H1], f32, tag="l1", name="pfold_6",
                            padded_shape=[128, NB])
            nc.tensor.matmul(pf6[0:KTAIL, :], lhs6, fv[:, t5, ABW:FW],
                             start=True, stop=True)
            nc.tensor.matmul(pf6[32:32 + KTAIL, 0:128], lhs6,
                             fv[:, t5, ABW + 128:ABW + 256],
                             start=True, stop=True, tile_position=(0, 32))
            nc.tensor.matmul(pf6[64:64 + KTAIL, 0:128], lhs6,
                             fv[:, t5, ABW:ABW + 128],
                             start=True, stop=True, tile_position=(0, 64))
            nc.tensor.matmul(pf6[96:96 + KTAIL, 0:128], lhs6,
                             fv[:, t5, ABW + 128:ABW + 256],
                             start=True, stop=True, tile_position=(0, 96))
            nc.vector.tensor_copy(w1eff6_sb[0:KTAIL, :], pf6[0:KTAIL, :])
            nc.vector.tensor_copy(w1eff6_sb[32:32 + KTAIL, 128:256],
                                  pf6[32:32 + KTAIL, 0:128])
            nc.vector.tensor_copy(w1eff6_sb[64:64 + KTAIL, 0:128],
                                  pf6[64:64 + KTAIL, 0:128])
            nc.vector.tensor_copy(w1eff6_sb[96:96 + KTAIL, 128:256],
                                  pf6[96:96 + KTAIL, 0:128])

            # ---- post-fold filler ----
            for wi in range(4):
                pw = psmtp.tile([128, NB], f32, tag="mt", name=f"pfill_{wi}")
                nc.tensor.matmul(pw[:], warm_sb[:, :128], warm_sb[:],
                                 start=True, stop=True)

            w1v = w1eff_sb[:].rearrange("p (k c) -> p k c", c=H1)
            w2v = w2_sb[:].rearrange("p (k c) -> p k c", c=H2)

            xviews = [None] * len(SUP_WIDTHS)
            emitted = [-1]
            psA, psB, pmt, h1t, h2t = {}, {}, {}, {}, {}

            def emit_super(s):
                xt = xpool.tile([128, 7 * SUP], bf16, tag="x", name=f"xt_{s}")
                xv = xt[:].rearrange("p (k c) -> p k c", c=SUP)
                xviews[s] = xv
                # garbage zones of the tail k-tile must be zero: the
                # zero-padded mt tail MM streams all 128 partitions. Done
                # per super so every logical tile has the region written.
                # (DVE partition starts must be 32-aligned, so clear the
                # whole k-tile; the tail DMAs below overwrite their rows.)
                nc.vector.memset(xv[:, 6, :], 0.0)
                sw = SUP_WIDTHS[s]
                c0 = _SUP_START[s]
                for g in range(4):
                    nc.sync.dma_start(
                        xv[32 * g:32 * g + KTAIL, 6, :sw],
                        xT_d[NKT * KT:U, c0:c0 + sw])
                nc.sync.dma_start(
                    xv[:, 0:6, :sw],
                    xT_d[0:NKT * KT, c0:c0 + sw]
                    .rearrange("(k p) c -> p k c", p=128))

            def ensure_supers(upto_block):
                s_needed = _BLK2SUP[min(upto_block, NBLK - 1)][0]
                while emitted[0] < s_needed:
                    emitted[0] += 1
                    emit_super(emitted[0])

            def xview(b):
                s, off = _BLK2SUP[b]
                return xviews[s], off

            def s1_block(b):
                # k-tail span: m0+m1 tails row-tiled (opens the psum groups)
                xv, off = xview(b)
                pA = ps1p.tile([128, NB], f32, tag="l1", name=f"pA_{b}")
                pB = ps1p.tile([128, NB], f32, tag="l1", name=f"pB_{b}")
                psA[b], psB[b] = pA, pB
                p0 = 0 if b % 2 == 0 else 64
                p1 = p0 + 32
                nc.tensor.matmul(pA[:], w1eff6_sb[p0:p0 + KTAIL, 0:128],
                                 xv[p0:p0 + KTAIL, 6, off:off + NB],
                                 start=True, stop=False)
                kw = {"tile_position": (96, 0)} if p1 == 96 else {}
                nc.tensor.matmul(pB[:], w1eff6_sb[p1:p1 + KTAIL, 128:256],
                                 xv[p1:p1 + KTAIL, 6, off:off + NB],
                                 start=True, stop=False, **kw)

            def s2_block(b):
                # 12 full 128x128 passes + relu drains for m0/m1
                xv, off = xview(b)
                pA, pB = psA[b], psB[b]
                h1 = hpool.tile([128, 3 * NB], bf16, tag="h1", name=f"h1_{b}")
                h1t[b] = h1
                h1v = h1[:].rearrange("p (k c) -> p k c", c=NB)
                # parts MT:128 of the mt third must be zero for the
                # zero-padded L2 k3 pass; cleared per block so every
                # logical tile has the region written (32-aligned start:
                # clear the whole third; the mt relu overwrites 0:MT).
                nc.vector.memset(h1v[:, 2, :], 0.0)
                for kt in range(NKT):
                    nc.tensor.matmul(pA[:], w1v[:, kt, 0:128],
                                     xv[:, kt, off:off + NB],
                                     start=False, stop=(kt == NKT - 1))
                nc.scalar.activation(h1v[:, 0, :], pA[:], relu,
                                     bias=bias_sb[:, 0:1], scale=1.0)
                for kt in range(NKT):
                    nc.tensor.matmul(pB[:], w1v[:, kt, 128:256],
                                     xv[:, kt, off:off + NB],
                                     start=False, stop=(kt == NKT - 1))
                nc.scalar.activation(h1v[:, 1, :], pB[:], relu,
                                     bias=bias_sb[:, 1:2], scale=1.0)

            def s3_pair(b0, b1):
                # mt (44 channels) split 22+22 across col tiles: 4 chains
                # (2 halves x 2 blocks) run concurrently in 128x32 mode at
                # PSUM partition slices 0:22 / 32:54 / 64:86 / 96:118, each
                # contracting all 7 k-tiles. The four chains are independent
                # per-partition accumulation groups in one bank; the bass
                # group checker only models bank-granular groups, so skip it.
                pm = psmtp.tile([128, NB], f32, tag="mt", name=f"pmt_{b0}")
                chains = []  # (psum base, block, w1 col lo/hi)
                for j, b in enumerate((b0, b0, b1, b1)):
                    lo = 256 + (j % 2) * MTH
                    chains.append((32 * j, b, lo, lo + MTH))
                for kt in range(NKT + 1):
                    for (pb_, b, lo, hi) in chains:
                        xv, off = xview(b)
                        if kt < NKT:
                            lhs = w1v[:, kt, lo:hi]
                        else:
                            lhs = w1eff6_sb[:, lo:hi]
                        nc.tensor.matmul(pm[pb_:pb_ + MTH, :], lhs,
                                         xv[:, min(kt, 6), off:off + NB],
                                         start=(kt == 0), stop=(kt == NKT),
                                         skip_group_check=True,
                                         tile_position=(0, pb_))
                for (pb_, b, lo, hi) in chains:
                    h1v = h1t[b][:].rearrange("p (k c) -> p k c", c=NB)
                    nc.scalar.activation(
                        h1v[pb_:pb_ + MTH, 2, :], pm[pb_:pb_ + MTH, :], relu,
                        bias=bias_sb[pb_:pb_ + MTH, 2:3], scale=1.0)

            def l2_block(b):
                h1v = h1t[b][:].rearrange("p (k c) -> p k c", c=NB)
                p2 = ps2p.tile([H2, NB], f32, tag="l2", name=f"p2_{b}",
                               padded_shape=[128, NB])
                for j in range(3):
                    # k3 weight chunk comes in even/odd-block variants that
                    # match where s3_pair parked the mt halves (0:22/32:54
                    # vs 64:86/96:118); all other rows are zero.
                    wj = j if j < 2 else 2 + (b % 2)
                    nc.tensor.matmul(p2[:], w2v[:, wj, :], h1v[:, j, :],
                                     start=(j == 0), stop=(j == 2))
                h2 = h2pool.tile([H2, NB], bf16, tag="h2", name=f"h2_{b}")
                h2t[b] = h2
                nc.vector.tensor_scalar(h2[:], p2[:], bias_sb[0:H2, 3:4], 0.0,
                                        alu_add, alu_max)

            def l3_group(g):
                # 4 blocks col-packed in 128x32 mode, then bias + store
                p3 = ps2p.tile([128, NB], f32, tag="l2", name=f"p3_{g}")
                y = ypool.tile([128, NB], f32, tag="y", name=f"y_{g}")
                for j in range(4):
                    nc.tensor.matmul(p3[32 * j:32 * j + H3, :], w3_sb[:],
                                     h2t[4 * g + j][:], start=True, stop=True,
                                     tile_position=(0, 32 * j))
                for j in range(4):
                    b = 4 * g + j
                    s, off = _BLK2SUP[b]
                    c0 = _SUP_START[s] + off
                    nc.vector.tensor_scalar_add(
                        y[32 * j:32 * j + H3, :], p3[32 * j:32 * j + H3, :],
                        bias_sb[32 * j:32 * j + H3, 4:5])
                    nc.gpsimd.dma_start(yT_d[:, c0:c0 + NB],
                                        y[32 * j:32 * j + H3, :])

            for p in range(NBLK // 2):
                b0, b1 = 2 * p, 2 * p + 1
                ensure_supers(b1 + 4)
                s1_block(b0)
                s1_block(b1)
                if p >= 1:
                    l2_block(b0 - 2)
                    l2_block(b1 - 2)
                s2_block(b0)
                s2_block(b1)
                s3_pair(b0, b1)
                if p >= 1 and (b1 - 2) % 4 == 3:
                    l3_group((b1 - 2) // 4)
            l2_block(NBLK - 2)
            l2_block(NBLK - 1)
            l3_group(NBLK // 4 - 1)

    nc.compile()
    return nc


def _build_amat_banded(conv_w: np.ndarray) -> np.ndarray:
    """Scatter the 9 conv weights into banded A^T [QP, ABW]."""
    amat = np.zeros((Q, U), np.float32)
    i = np.arange(26)
    j = np.arange(26)
    q = (26 * i[:, None] + j[None, :]).ravel()
    for ki in range(3):
        for kj in range(3):
            u = (28 * (i[:, None] + ki) + j[None, :] + kj).ravel()
            amat[q, u] = conv_w[ki, kj]
    banded = np.zeros((QP, ABW), np.float32)
    for (q0, p_real, c_lo, c_hi) in _fold_bands():
        w = min(U, 128 * c_hi) - 128 * c_lo
        banded[q0:q0 + p_real, :w] = \
            amat[q0:q0 + p_real, 128 * c_lo:128 * c_lo + w]
    return banded


def _make_in_maps(x, conv_w, W1, b1, W2, b2, W3, b3):
    bf = ml_dtypes.bfloat16
    xT = np.ascontiguousarray(np.asarray(x, np.float32).T.astype(bf))
    foldpk = np.zeros((QP, FW), np.float32)
    foldpk[:, :ABW] = _build_amat_banded(np.asarray(conv_w, np.float32))
    foldpk[:Q, ABW:] = np.asarray(W1, np.float32)
    foldpk = np.ascontiguousarray(foldpk.astype(bf))
    W2f = np.asarray(W2, np.float32)
    w2 = np.zeros((4 * KT, H2), np.float32)
    w2[0:H1 - MT] = W2f[0:H1 - MT]
    # k3 chunk, even-block variant: mt halves at parts 0:22 / 32:54
    w2[256 + 0:256 + MTH] = W2f[256:256 + MTH]
    w2[256 + 32:256 + 32 + MTH] = W2f[256 + MTH:H1]
    # odd-block variant: halves at parts 64:86 / 96:118
    w2[384 + 64:384 + 64 + MTH] = W2f[256:256 + MTH]
    w2[384 + 96:384 + 96 + MTH] = W2f[256 + MTH:H1]
    w2 = np.ascontiguousarray(w2.astype(bf))
    w3 = np.ascontiguousarray(np.asarray(W3, np.float32).astype(bf))
    bias = np.zeros((128, 5), np.float32)
    b1f = np.asarray(b1, np.float32)
    bias[:, 0] = b1f[0:128]
    bias[:, 1] = b1f[128:256]
    for j in range(4):
        lo = 256 + (j % 2) * MTH
        bias[32 * j:32 * j + MTH, 2] = b1f[lo:lo + MTH]
    bias[:H2, 3] = np.asarray(b2, np.float32)
    b3f = np.asarray(b3, np.float32)
    for j in range(4):
        bias[32 * j:32 * j + H3, 4] = b3f
    in_maps = []
    for c in range(N_CORES):
        in_maps.append({
            "xT": np.ascontiguousarray(xT[:, c * BC:(c + 1) * BC]),
            "fold": foldpk,
            "w2": w2, "w3": w3,
            "bias": bias,
        })
    return in_maps


def kernel(x, conv_w, W1, b1, W2, b2, W3, b3):
    x = np.asarray(x, dtype=np.float32)
    conv_w = np.asarray(conv_w, dtype=np.float32)

    if "nc" not in _prog_cache:
        _prog_cache["nc"] = _build_program()
    nc = _prog_cache["nc"]

    in_maps = _make_in_maps(x, conv_w, W1, b1, W2, b2, W3, b3)
    res = bass_utils.run_bass_kernel_spmd(
        nc, in_maps, core_ids=list(range(N_CORES))
    )

    out = np.empty((B, H3), np.float32)
    for c in range(N_CORES):
        out[c * BC:(c + 1) * BC, :] = res.results[c]["yT"].T
    return out
